# revision 17
# baseline (speedup 1.0000x reference)
"""Trainium2 Bass kernel for nn_EventEmulator (DVS event-camera emulator).

Reference computation per pixel (B*H*W independent pixels, F=16 frames):
  l_f = lin_log(frame_f)                       (linear below 20, log above)
  base_0 = l_0
  per step f=1..15:
    d = l_f - base
    p = floor(relu(d)/pt);  n = floor(relu(-d)/nt)
    base += p*pt - n*nt
    c_f = p - n
  voxel_k = sum_f w[f,k] * c_f                 (bilinear temporal weights, 5 bins)

v9 strategy (u-space scan, PSUM-resident state, batch+H sharding):
 - Shard (B=4) x (H halves) across 8 cores: core c gets batch c//2, rows
   256*(c%2) .. +256.  A frame slice [256, 512] is contiguous in DRAM ->
   ONE DMA per frame into a [128 x 1024] tile.
 - Work in u-space: u = lin_log(x)/pt.  Per-pixel g = pt/nt, a = nt/pt.
 - State: du_f = u_f - base_u lives in PSUM (one bank per 512-col chunk).
   The recurrence du_{f+1} = du_f - qd_f + (u_{f+1} - u_f) is applied by
   the PE with exact fp32 identity matmuls accumulating into the bank:
     mm(dub, -I, qd_f);  mm(dub, +I, du_next_delta)
   (fp32 PE matmul verified bit-exact for identity weights).
 - Per step per chunk:
     cnt = trunc-toward-zero(du>=0 ? du : du*g)
         = (select(du>=0, du-0.5, du*g+0.5) + M) - M   [custom DVE, fp16]
     qd  = select(cnt>=0, cnt, cnt*a)                  [custom DVE]
     PE: dub -= qd; dub += delta_u(next)               [2 fp32 matmuls]
     voxel psum[k] += w[k]*cnt                         [fp16 matmuls]
 - Frame prep: Ln on ACT, lin_log combine (custom) on DVE,
   u = l*ipt and delta_u = u_f - u_{f-1} on Pool.
 - Engine balance per step: DVE ~3.6us, Pool ~4.3us, PE ~4.1us, ACT ~2us.
"""

import os
import sys

for _p in ("/opt/trn_rl_repo", "/root/.axon_site/_ro/trn_rl_repo"):
    if os.path.isdir(_p):
        sys.path.insert(0, _p)
        break

import numpy as np

import concourse.bacc as bacc
import concourse.mybir as mybir
import concourse.tile as tile
from concourse import bass_utils
from concourse.dve_ops import (
    DveOp,
    OPS,
    CUSTOM_DVE_SPECS,
    _SUB_OPCODE_FOR_NAME,
    _CUSTOM_DVE_ROW_BASE,
)
from concourse.dve_spec import (
    Spec, Src0, Src1, C0, C1, C2, Zero, relu, maxx, minn, select, lower,
)
from concourse.dve_uop import DveOpSpec

# ---------------------------------------------------------------- constants
B, F, H, W = 4, 16, 512, 512
N_CORES = 8
HSH = H // 2                 # 256 rows per core (half of H), single batch
P = 128                      # partitions
NPIX = HSH * W               # 131072 pixels per core
NCOL = NPIX // P             # 1024
NCHUNK = 2
CW = NCOL // NCHUNK          # 512 columns per chunk

NUM_BINS = 5
MAGIC = 12582912.0           # 1.5 * 2^23: fp32 round-to-nearest-int magic
HALF = 0.5
F_LIN = float(np.float32(np.log(np.float32(20.0)) / np.float32(20.0)))
LN_BIAS = 1e-9
LN_CLAMP = 0.2               # any value in [0.1792, ln(20)] works
PREFETCH = 3

FP32 = mybir.dt.float32
FP16 = mybir.dt.float16


# ------------------------------------------------------- custom DVE ops
def _register_op(name, spec):
    """Register a custom DVE op at runtime (self-computed uops sha)."""
    for existing in OPS:
        if existing.name == name:
            return existing
    shas = {}
    for ver in ("v3", "v4"):
        s = DveOpSpec(name=name, opcode=0, uops=lower(spec, ver=ver), rd1_en=True)
        shas[ver] = s.sha(ver)
    op = DveOp(name, spec, subdim=False, uops_sha=shas)
    OPS.append(op)
    CUSTOM_DVE_SPECS[name] = spec
    _SUB_OPCODE_FOR_NAME[name] = _CUSTOM_DVE_ROW_BASE + len(OPS) - 1
    return op


def _np_magic_round(x):
    return (np.round(x.astype(np.float32) + MAGIC) - MAGIC).astype(np.float32)


# cnt = trunc-toward-zero(du >= 0 ? du : du*g) via shared magic-round chain:
#   pos: round(du - 0.5) = floor(du)   (ties at ints: measure zero)
#   neg: round(du*g + 0.5) = ceil(du*g) = -floor(-du*g)
# s1 = MAGIC, imm2 = 0.5; in1 = g = pt/nt per-pixel.
EVT_CNT = _register_op(
    "EVT_CNTV4",
    Spec(
        body=(select(Src0 >= Zero, Src0 - C2, Src0 * Src1 + C2) + C1) - C1,
        reference=lambda in0, in1, s0, s1, imm2: _np_magic_round(
            np.where(in0 >= 0, in0 - np.float32(0.5), in0 * in1 + np.float32(0.5))
        ),
    ),
)

# qd = cnt >= 0 ? cnt : cnt*a ; in1 = a = nt/pt per-pixel.
EVT_QD = _register_op(
    "EVT_QDV4",
    Spec(
        body=select(Src0 >= Zero, Src0, Src0 * Src1),
        reference=lambda in0, in1, s0, s1, imm2: np.where(
            in0 >= 0, in0, in0 * in1
        ).astype(np.float32),
    ),
)

# l = min(max(ln_x, s0), x * s1)   (lin_log combine)
EVT_LC = _register_op(
    "EVT_LC",
    Spec(
        body=minn(maxx(Src0, C0), Src1 * C1),
        reference=lambda in0, in1, s0, s1, imm2: np.minimum(
            np.maximum(in0, s0), in1 * s1
        ).astype(np.float32),
    ),
)


# ------------------------------------------------------- temporal weights
def _weight_table():
    """Per frame f=1..15: list of (slot, bin, w). slot indexes the diag tensor."""
    t = np.linspace(np.float32(0.0), np.float32(NUM_BINS - 1), F, dtype=np.float32)[1:]
    bins = np.arange(NUM_BINS, dtype=np.float32)
    wts = np.maximum(0.0, 1.0 - np.abs(t[:, None] - bins[None, :])).astype(np.float32)
    table = []  # [(f, [(slot,k,w), ...])]
    slots = []  # w value per slot
    for fi in range(15):
        touches = []
        for k in range(NUM_BINS):
            w = float(wts[fi, k])
            if w > 0.0:
                touches.append((len(slots), k, w))
                slots.append(w)
        table.append(touches)
    return table, slots


W_TABLE, W_SLOTS = _weight_table()
N_SLOTS = len(W_SLOTS)  # 29

# last frame index (0-based step index into W_TABLE) touching each bin
BIN_LAST = {}
for fi, touches in enumerate(W_TABLE):
    for _, k, _ in touches:
        BIN_LAST[k] = fi


def _diag_host():
    d = np.zeros((P, N_SLOTS * P), dtype=np.float16)
    eye = np.eye(P, dtype=np.float16)
    for j, w in enumerate(W_SLOTS):
        d[:, j * P : (j + 1) * P] = (np.float16(w) * eye).astype(np.float16)
    return d


# ------------------------------------------------------------ build kernel
_CACHED_NC = None


def _build_nc():
    nc = bacc.Bacc(
        "TRN2",
        target_bir_lowering=False,
        debug=False,
        enable_asserts=False,
        num_devices=N_CORES,
    )

    frames = nc.dram_tensor("frames", [F, HSH, W], FP32, kind="ExternalInput").ap()
    pos = nc.dram_tensor("pos", [HSH, W], FP32, kind="ExternalInput").ap()
    neg = nc.dram_tensor("neg", [HSH, W], FP32, kind="ExternalInput").ap()
    diag = nc.dram_tensor("diag", [P, N_SLOTS * P], FP16, kind="ExternalInput").ap()
    eyes = nc.dram_tensor("eyes", [P, 2 * P], FP32, kind="ExternalInput").ap()
    vox = nc.dram_tensor("vox", [NUM_BINS, HSH, W], FP32, kind="ExternalOutput").ap()

    ALU = mybir.AluOpType

    with tile.TileContext(nc) as tc:
        with (
            tc.tile_pool(name="const", bufs=1) as cpool,
            tc.tile_pool(name="frames", bufs=4) as xpool,
            tc.tile_pool(name="u", bufs=4) as upool,
            tc.tile_pool(name="tmp", bufs=4) as tpool,
            tc.tile_pool(name="stage", bufs=2) as stpool,
            tc.tile_pool(name="vox", bufs=2, space="PSUM") as vpool,
            tc.tile_pool(name="dstate", bufs=1, space="PSUM") as dpool,
        ):
            # ---------------- constants / thresholds prep
            lnb = cpool.tile([P, 1], FP32, tag="lnb")
            nc.vector.memset(lnb[:], LN_BIAS)
            # warm the ACT Ln function table immediately (the table load is
            # ~1.3us and would otherwise delay the first frame's Ln).
            atl = cpool.tile([P, 1], FP32, tag="atl")
            nc.scalar.activation(
                atl[:], lnb[:], mybir.ActivationFunctionType.Ln, bias=lnb[:], scale=1.0
            )

            # DMA order: frames 0/1 first (they gate the Ln->LC->u chain),
            # then thresholds, then PE-warmup identities and the diag weights.
            x0 = xpool.tile([P, NCOL], FP32, tag="x", bufs=4, name="x0")
            nc.sync.dma_start(x0[:], frames[0].rearrange("(hh h) w -> hh (h w)", h=2))
            x1 = xpool.tile([P, NCOL], FP32, tag="x", bufs=4, name="x1")
            nc.sync.dma_start(x1[:], frames[1].rearrange("(hh h) w -> hh (h w)", h=2))

            pt = cpool.tile([P, NCOL], FP32, tag="pt")
            nt = cpool.tile([P, NCOL], FP32, tag="nt")
            nc.sync.dma_start(pt[:], pos.rearrange("(hh h) w -> hh (h w)", h=2))
            nc.sync.dma_start(nt[:], neg.rearrange("(hh h) w -> hh (h w)", h=2))

            eye = cpool.tile([P, 2 * P], FP32, tag="eyes")
            nc.sync.dma_start(eye[:], eyes[:])
            IP = eye[:, 0:P]        # +identity
            IM = eye[:, P : 2 * P]  # -identity

            dg = cpool.tile([P, N_SLOTS * P], FP16, tag="diag")
            nc.sync.dma_start(dg[:], diag[:])

            # PE p-state warmup: ~3us of continuous matmuls brings the tensor
            # engine to full clock before the state matmuls start.
            warm = dpool.tile([P, P], FP32, tag="warm", name="warmps")
            for _ in range(28):
                nc.tensor.matmul(
                    warm[:], IP, IP, start=True, stop=True, skip_group_check=True
                )

            ipt = cpool.tile([P, NCOL], FP32, tag="ipt")
            int_ = cpool.tile([P, NCOL], FP32, tag="int")
            rscr = cpool.tile([P, NCOL], FP32, tag="rscr")
            rscr2 = cpool.tile([P, NCOL], FP32, tag="rscr2")
            nc.vector.reciprocal_approx_accurate(ipt[:], pt[:], rscr[:])
            nc.vector.reciprocal_approx_accurate(int_[:], nt[:], rscr2[:])
            # g = pt/nt, a = nt/pt (per pixel), chunk-split across DVE/Pool:
            # the c0 halves come first (first CNT/QOP read them earliest).
            g = cpool.tile([P, NCOL], FP32, tag="g")
            a = cpool.tile([P, NCOL], FP32, tag="a")
            nc.vector.tensor_tensor(g[:, 0:CW], pt[:, 0:CW], int_[:, 0:CW], ALU.mult)
            nc.vector.tensor_tensor(a[:, 0:CW], nt[:, 0:CW], ipt[:, 0:CW], ALU.mult)
            nc.gpsimd.tensor_tensor(
                g[:, CW:NCOL], pt[:, CW:NCOL], int_[:, CW:NCOL], ALU.mult
            )
            nc.gpsimd.tensor_tensor(
                a[:, CW:NCOL], nt[:, CW:NCOL], ipt[:, CW:NCOL], ALU.mult
            )

            # ---------------- frame prep: x -> lnx -> l -> u -> delta_u
            def prep_frame(f, x=None):
                if x is None:
                    x = xpool.tile([P, NCOL], FP32, tag="x", bufs=4, name=f"x{f}")
                    nc.sync.dma_start(
                        x[:], frames[f].rearrange("(hh h) w -> hh (h w)", h=2)
                    )
                lnx = xpool.tile([P, NCOL], FP32, tag="lnx", bufs=3, name=f"lnx{f}")
                nc.scalar.activation(
                    lnx[:], x[:],
                    mybir.ActivationFunctionType.Ln, bias=lnb[:], scale=1.0,
                )
                l = xpool.tile([P, NCOL], FP32, tag="l", bufs=3, name=f"l{f}")
                nc.vector._custom_dve(
                    EVT_LC, out=l[:], in0=lnx[:], in1=x[:], s0=LN_CLAMP, s1=F_LIN,
                )
                u = upool.tile([P, NCOL], FP32, tag="u", bufs=3, name=f"u{f}")
                nc.gpsimd.tensor_tensor(u[:], l[:], ipt[:], ALU.mult)
                return u

            def delta_u(f, u_prev, u_new):
                # split across DVE (c0) / Pool (c1) to balance engine load
                duv = upool.tile([P, NCOL], FP32, tag="duv", bufs=4, name=f"duv{f}")
                nc.vector.tensor_tensor(
                    duv[:, 0:CW], u_new[:, 0:CW], u_prev[:, 0:CW], ALU.subtract
                )
                nc.gpsimd.tensor_tensor(
                    duv[:, CW:NCOL], u_new[:, CW:NCOL], u_prev[:, CW:NCOL],
                    ALU.subtract,
                )
                return duv

            # ---------------- psum bins
            vox_psum = {}     # (k, c) -> psum tile
            mm_count = {}     # (k, c) -> matmuls emitted so far
            MM_TOTAL = {}     # (k, c) -> total matmuls that will be emitted
            for k in range(NUM_BINS):
                n_frames = sum(
                    1 for touches in W_TABLE for (_, kk, _) in touches if kk == k
                )
                for c in range(NCHUNK):
                    MM_TOTAL[(k, c)] = n_frames

            def bin_matmul(k, c, w_slot, rhs_ap):
                key = (k, c)
                if key not in vox_psum:
                    vox_psum[key] = vpool.tile(
                        [P, CW], FP32, tag=f"vox{c}", name=f"voxp{k}_{c}"
                    )
                    mm_count[key] = 0
                first = mm_count[key] == 0
                last = mm_count[key] == MM_TOTAL[key] - 1
                nc.tensor.matmul(
                    vox_psum[key][:],
                    dg[:, w_slot * P : (w_slot + 1) * P],
                    rhs_ap,
                    start=first,
                    stop=last,
                )
                mm_count[key] += 1

            def bin_flush(k, c):
                # psum -> sbuf -> dram; chunk c covers h-row c of each hh pair
                st = stpool.tile([P, CW], FP32, tag=f"stage{c}", name=f"st{k}_{c}")
                nc.scalar.copy(st[:], vox_psum[(k, c)][:])
                nc.sync.dma_start(
                    vox[k].rearrange("(hh h) w -> h hh w", h=2)[c], st[:]
                )

            # ---------------- prep frames 0..1+PREFETCH, state init
            # Frames 0/1: compute u and delta_u_1 at 512-chunk granularity on
            # both engines so the chunk-0 scan can start as early as possible.
            def prep_uv_split(f, x):
                lnx = xpool.tile([P, NCOL], FP32, tag="lnx", bufs=3, name=f"lnx{f}")
                nc.scalar.activation(
                    lnx[:], x[:],
                    mybir.ActivationFunctionType.Ln, bias=lnb[:], scale=1.0,
                )
                l = xpool.tile([P, NCOL], FP32, tag="l", bufs=3, name=f"l{f}")
                nc.vector._custom_dve(
                    EVT_LC, out=l[:], in0=lnx[:], in1=x[:], s0=LN_CLAMP, s1=F_LIN,
                )
                u = upool.tile([P, NCOL], FP32, tag="u", bufs=3, name=f"u{f}")
                nc.vector.tensor_tensor(
                    u[:, 0:CW], l[:, 0:CW], ipt[:, 0:CW], ALU.mult
                )
                nc.gpsimd.tensor_tensor(
                    u[:, CW:NCOL], l[:, CW:NCOL], ipt[:, CW:NCOL], ALU.mult
                )
                return u

            u_tiles = {0: prep_uv_split(0, x0), 1: prep_uv_split(1, x1)}
            duv_tiles = {1: delta_u(1, u_tiles[0], u_tiles[1])}

            # du state per chunk in PSUM: init du_1 = delta_u_1 (c0 first)
            dub = {}
            for c in range(NCHUNK):
                cols = slice(c * CW, (c + 1) * CW)
                dub[c] = dpool.tile([P, CW], FP32, tag=f"dub{c}", name=f"dub{c}")
                nc.tensor.matmul(
                    dub[c][:], IP, duv_tiles[1][:, cols],
                    start=True, stop=True, skip_group_check=True,
                )

            for f in range(2, 2 + PREFETCH):
                u_tiles[f] = prep_frame(f)
                duv_tiles[f] = delta_u(f, u_tiles[f - 1], u_tiles[f])

            # ---------------- scan
            for fi in range(15):
                f = fi + 1
                touches = W_TABLE[fi]

                cnt = {}
                for c in range(NCHUNK):
                    cols = slice(c * CW, (c + 1) * CW)
                    cntc = tpool.tile([P, CW], FP16, tag=f"c{c}", name=f"cnt{f}_{c}")
                    nc.vector._custom_dve(
                        EVT_CNT, out=cntc[:], in0=dub[c][:], in1=g[:, cols],
                        s1=MAGIC, imm2=HALF,
                    )
                    cnt[c] = cntc
                    if fi < 14:
                        qd = tpool.tile([P, CW], FP32, tag=f"q{c}", name=f"qd{f}_{c}")
                        nc.vector._custom_dve(
                            EVT_QD, out=qd[:], in0=cntc[:], in1=a[:, cols],
                        )
                        # PE: du -= qd ; du += delta_u(f+1)   (exact fp32)
                        nc.tensor.matmul(
                            dub[c][:], IM, qd[:],
                            start=False, stop=True, skip_group_check=True,
                        )
                        nc.tensor.matmul(
                            dub[c][:], IP, duv_tiles[f + 1][:, cols],
                            start=False, stop=True, skip_group_check=True,
                        )

                for c in range(NCHUNK):
                    for slot, k, _w in touches:
                        bin_matmul(k, c, slot, cnt[c][:])
                    # flush any bin whose last touch was this chunk's matmul
                    for k in range(NUM_BINS):
                        if BIN_LAST[k] == fi:
                            bin_flush(k, c)

                # prep AFTER the scan ops (keeps next frames' LC/u/delta off
                # the in-order critical path of DVE/Pool).
                fp = f + 1 + PREFETCH
                if fp <= 15:
                    u_tiles[fp] = prep_frame(fp)
                    duv_tiles[fp] = delta_u(fp, u_tiles[fp - 1], u_tiles[fp])
                    del u_tiles[fp - 1]

    nc.compile()
    return nc


def _get_nc():
    global _CACHED_NC
    if _CACHED_NC is None:
        _CACHED_NC = _build_nc()
    return _CACHED_NC


def _eyes_host():
    e = np.eye(P, dtype=np.float32)
    return np.concatenate([e, -e], axis=1)


# ------------------------------------------------------------------ driver
def kernel(frames, t_frames, pos_thres, neg_thres):
    frames = np.asarray(frames, dtype=np.float32)
    pos_thres = np.asarray(pos_thres, dtype=np.float32)
    neg_thres = np.asarray(neg_thres, dtype=np.float32)

    nc = _get_nc()
    dg = _diag_host()
    ey = _eyes_host()
    in_maps = []
    for c in range(N_CORES):
        b = c // 2
        hs = slice((c % 2) * HSH, (c % 2 + 1) * HSH)
        in_maps.append(
            {
                "frames": np.ascontiguousarray(frames[b, :, hs, :]),
                "pos": np.ascontiguousarray(pos_thres[b, 0, hs, :]),
                "neg": np.ascontiguousarray(neg_thres[b, 0, hs, :]),
                "diag": dg,
                "eyes": ey,
            }
        )

    res = bass_utils.run_bass_kernel_spmd(nc, in_maps, core_ids=list(range(N_CORES)))
    out = np.empty((B, NUM_BINS, H, W), dtype=np.float32)
    for c in range(N_CORES):
        b = c // 2
        hs = slice((c % 2) * HSH, (c % 2 + 1) * HSH)
        out[b, :, hs, :] = res.results[c]["vox"]
    return out


# revision 18
# speedup vs baseline: 1.0099x; 1.0099x over previous
"""Trainium2 Bass kernel for nn_EventEmulator (DVS event-camera emulator).

Reference computation per pixel (B*H*W independent pixels, F=16 frames):
  l_f = lin_log(frame_f)                       (linear below 20, log above)
  base_0 = l_0
  per step f=1..15:
    d = l_f - base
    p = floor(relu(d)/pt);  n = floor(relu(-d)/nt)
    base += p*pt - n*nt
    c_f = p - n
  voxel_k = sum_f w[f,k] * c_f                 (bilinear temporal weights, 5 bins)

v9 strategy (u-space scan, PSUM-resident state, batch+H sharding):
 - Shard (B=4) x (H halves) across 8 cores: core c gets batch c//2, rows
   256*(c%2) .. +256.  A frame slice [256, 512] is contiguous in DRAM ->
   ONE DMA per frame into a [128 x 1024] tile.
 - Work in u-space: u = lin_log(x)/pt.  Per-pixel g = pt/nt, a = nt/pt.
 - State: du_f = u_f - base_u lives in PSUM (one bank per 512-col chunk).
   The recurrence du_{f+1} = du_f - qd_f + (u_{f+1} - u_f) is applied by
   the PE with exact fp32 identity matmuls accumulating into the bank:
     mm(dub, -I, qd_f);  mm(dub, +I, du_next_delta)
   (fp32 PE matmul verified bit-exact for identity weights).
 - Per step per chunk:
     cnt = trunc-toward-zero(du>=0 ? du : du*g)
         = (select(du>=0, du-0.5, du*g+0.5) + M) - M   [custom DVE, fp16]
     qd  = select(cnt>=0, cnt, cnt*a)                  [custom DVE]
     PE: dub -= qd; dub += delta_u(next)               [2 fp32 matmuls]
     voxel psum[k] += w[k]*cnt                         [fp16 matmuls]
 - Frame prep: Ln on ACT, lin_log combine (custom) on DVE,
   u = l*ipt and delta_u = u_f - u_{f-1} on Pool.
 - Engine balance per step: DVE ~3.6us, Pool ~4.3us, PE ~4.1us, ACT ~2us.
"""

import os
import sys

for _p in ("/opt/trn_rl_repo", "/root/.axon_site/_ro/trn_rl_repo"):
    if os.path.isdir(_p):
        sys.path.insert(0, _p)
        break

import numpy as np

import concourse.bacc as bacc
import concourse.mybir as mybir
import concourse.tile as tile
from concourse import bass_utils
from concourse.dve_ops import (
    DveOp,
    OPS,
    CUSTOM_DVE_SPECS,
    _SUB_OPCODE_FOR_NAME,
    _CUSTOM_DVE_ROW_BASE,
)
from concourse.dve_spec import (
    Spec, Src0, Src1, C0, C1, C2, Zero, relu, maxx, minn, select, lower,
)
from concourse.dve_uop import DveOpSpec

# ---------------------------------------------------------------- constants
B, F, H, W = 4, 16, 512, 512
N_CORES = 8
HSH = H // 2                 # 256 rows per core (half of H), single batch
P = 128                      # partitions
NPIX = HSH * W               # 131072 pixels per core
NCOL = NPIX // P             # 1024
NCHUNK = 2
CW = NCOL // NCHUNK          # 512 columns per chunk

NUM_BINS = 5
MAGIC = 12582912.0           # 1.5 * 2^23: fp32 round-to-nearest-int magic
HALF = 0.5
F_LIN = float(np.float32(np.log(np.float32(20.0)) / np.float32(20.0)))
LN_BIAS = 1e-9
LN_CLAMP = 0.2               # any value in [0.1792, ln(20)] works
PREFETCH = 3

FP32 = mybir.dt.float32
FP16 = mybir.dt.float16


# ------------------------------------------------------- custom DVE ops
def _register_op(name, spec):
    """Register a custom DVE op at runtime (self-computed uops sha)."""
    for existing in OPS:
        if existing.name == name:
            return existing
    shas = {}
    for ver in ("v3", "v4"):
        s = DveOpSpec(name=name, opcode=0, uops=lower(spec, ver=ver), rd1_en=True)
        shas[ver] = s.sha(ver)
    op = DveOp(name, spec, subdim=False, uops_sha=shas)
    OPS.append(op)
    CUSTOM_DVE_SPECS[name] = spec
    _SUB_OPCODE_FOR_NAME[name] = _CUSTOM_DVE_ROW_BASE + len(OPS) - 1
    return op


def _np_magic_round(x):
    return (np.round(x.astype(np.float32) + MAGIC) - MAGIC).astype(np.float32)


# cnt = trunc-toward-zero(du >= 0 ? du : du*g) via shared magic-round chain:
#   pos: round(du - 0.5) = floor(du)   (ties at ints: measure zero)
#   neg: round(du*g + 0.5) = ceil(du*g) = -floor(-du*g)
# s1 = MAGIC, imm2 = 0.5; in1 = g = pt/nt per-pixel.
EVT_CNT = _register_op(
    "EVT_CNTV4",
    Spec(
        body=(select(Src0 >= Zero, Src0 - C2, Src0 * Src1 + C2) + C1) - C1,
        reference=lambda in0, in1, s0, s1, imm2: _np_magic_round(
            np.where(in0 >= 0, in0 - np.float32(0.5), in0 * in1 + np.float32(0.5))
        ),
    ),
)

# qd = cnt >= 0 ? cnt : cnt*a ; in1 = a = nt/pt per-pixel.
EVT_QD = _register_op(
    "EVT_QDV4",
    Spec(
        body=select(Src0 >= Zero, Src0, Src0 * Src1),
        reference=lambda in0, in1, s0, s1, imm2: np.where(
            in0 >= 0, in0, in0 * in1
        ).astype(np.float32),
    ),
)

# l = min(max(ln_x, s0), x * s1)   (lin_log combine)
EVT_LC = _register_op(
    "EVT_LC",
    Spec(
        body=minn(maxx(Src0, C0), Src1 * C1),
        reference=lambda in0, in1, s0, s1, imm2: np.minimum(
            np.maximum(in0, s0), in1 * s1
        ).astype(np.float32),
    ),
)


# ------------------------------------------------------- temporal weights
def _weight_table():
    """Per frame f=1..15: list of (slot, bin, w). slot indexes the diag tensor."""
    t = np.linspace(np.float32(0.0), np.float32(NUM_BINS - 1), F, dtype=np.float32)[1:]
    bins = np.arange(NUM_BINS, dtype=np.float32)
    wts = np.maximum(0.0, 1.0 - np.abs(t[:, None] - bins[None, :])).astype(np.float32)
    table = []  # [(f, [(slot,k,w), ...])]
    slots = []  # w value per slot
    for fi in range(15):
        touches = []
        for k in range(NUM_BINS):
            w = float(wts[fi, k])
            if w > 0.0:
                touches.append((len(slots), k, w))
                slots.append(w)
        table.append(touches)
    return table, slots


W_TABLE, W_SLOTS = _weight_table()
N_SLOTS = len(W_SLOTS)  # 29

# last frame index (0-based step index into W_TABLE) touching each bin
BIN_LAST = {}
for fi, touches in enumerate(W_TABLE):
    for _, k, _ in touches:
        BIN_LAST[k] = fi


def _diag_host():
    d = np.zeros((P, N_SLOTS * P), dtype=np.float16)
    eye = np.eye(P, dtype=np.float16)
    for j, w in enumerate(W_SLOTS):
        d[:, j * P : (j + 1) * P] = (np.float16(w) * eye).astype(np.float16)
    return d


# ------------------------------------------------------------ build kernel
_CACHED_NC = None


def _build_nc():
    nc = bacc.Bacc(
        "TRN2",
        target_bir_lowering=False,
        debug=False,
        enable_asserts=False,
        num_devices=N_CORES,
    )

    frames = nc.dram_tensor("frames", [F, HSH, W], FP32, kind="ExternalInput").ap()
    pos = nc.dram_tensor("pos", [HSH, W], FP32, kind="ExternalInput").ap()
    neg = nc.dram_tensor("neg", [HSH, W], FP32, kind="ExternalInput").ap()
    diag = nc.dram_tensor("diag", [P, N_SLOTS * P], FP16, kind="ExternalInput").ap()
    eyes = nc.dram_tensor("eyes", [P, 2 * P], FP32, kind="ExternalInput").ap()
    vox = nc.dram_tensor("vox", [NUM_BINS, HSH, W], FP32, kind="ExternalOutput").ap()

    ALU = mybir.AluOpType

    with tile.TileContext(nc) as tc:
        with (
            tc.tile_pool(name="const", bufs=1) as cpool,
            tc.tile_pool(name="frames", bufs=4) as xpool,
            tc.tile_pool(name="u", bufs=4) as upool,
            tc.tile_pool(name="tmp", bufs=4) as tpool,
            tc.tile_pool(name="stage", bufs=2) as stpool,
            tc.tile_pool(name="vox", bufs=2, space="PSUM") as vpool,
            tc.tile_pool(name="dstate", bufs=1, space="PSUM") as dpool,
        ):
            # ---------------- constants / thresholds prep
            lnb = cpool.tile([P, 1], FP32, tag="lnb")
            nc.vector.memset(lnb[:], LN_BIAS)
            # warm the ACT Ln function table immediately (the table load is
            # ~1.3us and would otherwise delay the first frame's Ln).
            atl = cpool.tile([P, 1], FP32, tag="atl")
            nc.scalar.activation(
                atl[:], lnb[:], mybir.ActivationFunctionType.Ln, bias=lnb[:], scale=1.0
            )

            # DMA order: frames 0/1 first (they gate the Ln->LC->u chain),
            # then thresholds, then PE-warmup identities and the diag weights.
            x0 = xpool.tile([P, NCOL], FP32, tag="x", bufs=4, name="x0")
            nc.sync.dma_start(x0[:], frames[0].rearrange("(hh h) w -> hh (h w)", h=2))
            x1 = xpool.tile([P, NCOL], FP32, tag="x", bufs=4, name="x1")
            nc.sync.dma_start(x1[:], frames[1].rearrange("(hh h) w -> hh (h w)", h=2))

            pt = cpool.tile([P, NCOL], FP32, tag="pt")
            nt = cpool.tile([P, NCOL], FP32, tag="nt")
            nc.sync.dma_start(pt[:], pos.rearrange("(hh h) w -> hh (h w)", h=2))
            nc.sync.dma_start(nt[:], neg.rearrange("(hh h) w -> hh (h w)", h=2))

            eye = cpool.tile([P, 2 * P], FP32, tag="eyes")
            nc.sync.dma_start(eye[:], eyes[:])
            IP = eye[:, 0:P]        # +identity
            IM = eye[:, P : 2 * P]  # -identity

            dg = cpool.tile([P, N_SLOTS * P], FP16, tag="diag")
            nc.sync.dma_start(dg[:], diag[:])

            ipt = cpool.tile([P, NCOL], FP32, tag="ipt")
            int_ = cpool.tile([P, NCOL], FP32, tag="int")
            rscr = cpool.tile([P, NCOL], FP32, tag="rscr")
            rscr2 = cpool.tile([P, NCOL], FP32, tag="rscr2")
            nc.vector.reciprocal_approx_accurate(ipt[:], pt[:], rscr[:])
            nc.vector.reciprocal_approx_accurate(int_[:], nt[:], rscr2[:])
            # g = pt/nt, a = nt/pt (per pixel), chunk-split across DVE/Pool:
            # the c0 halves come first (first CNT/QOP read them earliest).
            g = cpool.tile([P, NCOL], FP32, tag="g")
            a = cpool.tile([P, NCOL], FP32, tag="a")
            nc.vector.tensor_tensor(g[:, 0:CW], pt[:, 0:CW], int_[:, 0:CW], ALU.mult)
            nc.vector.tensor_tensor(a[:, 0:CW], nt[:, 0:CW], ipt[:, 0:CW], ALU.mult)
            nc.gpsimd.tensor_tensor(
                g[:, CW:NCOL], pt[:, CW:NCOL], int_[:, CW:NCOL], ALU.mult
            )
            nc.gpsimd.tensor_tensor(
                a[:, CW:NCOL], nt[:, CW:NCOL], ipt[:, CW:NCOL], ALU.mult
            )

            # ---------------- frame prep: x -> lnx -> l -> u -> delta_u
            def prep_frame(f, x=None):
                if x is None:
                    x = xpool.tile([P, NCOL], FP32, tag="x", bufs=4, name=f"x{f}")
                    nc.sync.dma_start(
                        x[:], frames[f].rearrange("(hh h) w -> hh (h w)", h=2)
                    )
                lnx = xpool.tile([P, NCOL], FP32, tag="lnx", bufs=3, name=f"lnx{f}")
                nc.scalar.activation(
                    lnx[:], x[:],
                    mybir.ActivationFunctionType.Ln, bias=lnb[:], scale=1.0,
                )
                l = xpool.tile([P, NCOL], FP32, tag="l", bufs=3, name=f"l{f}")
                nc.vector._custom_dve(
                    EVT_LC, out=l[:], in0=lnx[:], in1=x[:], s0=LN_CLAMP, s1=F_LIN,
                )
                u = upool.tile([P, NCOL], FP32, tag="u", bufs=3, name=f"u{f}")
                nc.gpsimd.tensor_tensor(u[:], l[:], ipt[:], ALU.mult)
                return u

            def delta_u(f, u_prev, u_new):
                # split across DVE (c0) / Pool (c1) to balance engine load
                duv = upool.tile([P, NCOL], FP32, tag="duv", bufs=4, name=f"duv{f}")
                nc.vector.tensor_tensor(
                    duv[:, 0:CW], u_new[:, 0:CW], u_prev[:, 0:CW], ALU.subtract
                )
                nc.gpsimd.tensor_tensor(
                    duv[:, CW:NCOL], u_new[:, CW:NCOL], u_prev[:, CW:NCOL],
                    ALU.subtract,
                )
                return duv

            # ---------------- psum bins
            vox_psum = {}     # (k, c) -> psum tile
            mm_count = {}     # (k, c) -> matmuls emitted so far
            MM_TOTAL = {}     # (k, c) -> total matmuls that will be emitted
            for k in range(NUM_BINS):
                n_frames = sum(
                    1 for touches in W_TABLE for (_, kk, _) in touches if kk == k
                )
                for c in range(NCHUNK):
                    MM_TOTAL[(k, c)] = n_frames

            def bin_matmul(k, c, w_slot, rhs_ap):
                key = (k, c)
                if key not in vox_psum:
                    vox_psum[key] = vpool.tile(
                        [P, CW], FP32, tag=f"vox{c}", name=f"voxp{k}_{c}"
                    )
                    mm_count[key] = 0
                first = mm_count[key] == 0
                last = mm_count[key] == MM_TOTAL[key] - 1
                nc.tensor.matmul(
                    vox_psum[key][:],
                    dg[:, w_slot * P : (w_slot + 1) * P],
                    rhs_ap,
                    start=first,
                    stop=last,
                )
                mm_count[key] += 1

            def bin_flush(k, c):
                # psum -> sbuf -> dram; chunk c covers h-row c of each hh pair
                st = stpool.tile([P, CW], FP32, tag=f"stage{c}", name=f"st{k}_{c}")
                nc.scalar.copy(st[:], vox_psum[(k, c)][:])
                nc.sync.dma_start(
                    vox[k].rearrange("(hh h) w -> h hh w", h=2)[c], st[:]
                )

            # ---------------- prep frames 0..1+PREFETCH, state init
            # Frames 0/1: compute u and delta_u_1 at 512-chunk granularity on
            # both engines so the chunk-0 scan can start as early as possible.
            def prep_uv_split(f, x):
                lnx = xpool.tile([P, NCOL], FP32, tag="lnx", bufs=3, name=f"lnx{f}")
                nc.scalar.activation(
                    lnx[:], x[:],
                    mybir.ActivationFunctionType.Ln, bias=lnb[:], scale=1.0,
                )
                l = xpool.tile([P, NCOL], FP32, tag="l", bufs=3, name=f"l{f}")
                nc.vector._custom_dve(
                    EVT_LC, out=l[:], in0=lnx[:], in1=x[:], s0=LN_CLAMP, s1=F_LIN,
                )
                u = upool.tile([P, NCOL], FP32, tag="u", bufs=3, name=f"u{f}")
                nc.vector.tensor_tensor(
                    u[:, 0:CW], l[:, 0:CW], ipt[:, 0:CW], ALU.mult
                )
                nc.gpsimd.tensor_tensor(
                    u[:, CW:NCOL], l[:, CW:NCOL], ipt[:, CW:NCOL], ALU.mult
                )
                return u

            u_tiles = {0: prep_uv_split(0, x0), 1: prep_uv_split(1, x1)}
            duv_tiles = {1: delta_u(1, u_tiles[0], u_tiles[1])}

            # du state per chunk in PSUM: init du_1 = delta_u_1 (c0 first)
            dub = {}
            for c in range(NCHUNK):
                cols = slice(c * CW, (c + 1) * CW)
                dub[c] = dpool.tile([P, CW], FP32, tag=f"dub{c}", name=f"dub{c}")
                nc.tensor.matmul(
                    dub[c][:], IP, duv_tiles[1][:, cols],
                    start=True, stop=True, skip_group_check=True,
                )

            for f in range(2, 2 + PREFETCH):
                u_tiles[f] = prep_frame(f)
                duv_tiles[f] = delta_u(f, u_tiles[f - 1], u_tiles[f])

            # ---------------- scan
            for fi in range(15):
                f = fi + 1
                touches = W_TABLE[fi]

                cnt = {}
                for c in range(NCHUNK):
                    cols = slice(c * CW, (c + 1) * CW)
                    cntc = tpool.tile([P, CW], FP16, tag=f"c{c}", name=f"cnt{f}_{c}")
                    nc.vector._custom_dve(
                        EVT_CNT, out=cntc[:], in0=dub[c][:], in1=g[:, cols],
                        s1=MAGIC, imm2=HALF,
                    )
                    cnt[c] = cntc
                    if fi < 14:
                        qd = tpool.tile([P, CW], FP32, tag=f"q{c}", name=f"qd{f}_{c}")
                        nc.vector._custom_dve(
                            EVT_QD, out=qd[:], in0=cntc[:], in1=a[:, cols],
                        )
                        # PE: du -= qd ; du += delta_u(f+1)   (exact fp32)
                        nc.tensor.matmul(
                            dub[c][:], IM, qd[:],
                            start=False, stop=True, skip_group_check=True,
                        )
                        nc.tensor.matmul(
                            dub[c][:], IP, duv_tiles[f + 1][:, cols],
                            start=False, stop=True, skip_group_check=True,
                        )

                for c in range(NCHUNK):
                    for slot, k, _w in touches:
                        bin_matmul(k, c, slot, cnt[c][:])
                    # flush any bin whose last touch was this chunk's matmul
                    for k in range(NUM_BINS):
                        if BIN_LAST[k] == fi:
                            bin_flush(k, c)

                # prep AFTER the scan ops (keeps next frames' LC/u/delta off
                # the in-order critical path of DVE/Pool).
                fp = f + 1 + PREFETCH
                if fp <= 15:
                    u_tiles[fp] = prep_frame(fp)
                    duv_tiles[fp] = delta_u(fp, u_tiles[fp - 1], u_tiles[fp])
                    del u_tiles[fp - 1]

    nc.compile()
    return nc


def _get_nc():
    global _CACHED_NC
    if _CACHED_NC is None:
        _CACHED_NC = _build_nc()
    return _CACHED_NC


def _eyes_host():
    e = np.eye(P, dtype=np.float32)
    return np.concatenate([e, -e], axis=1)


# ------------------------------------------------------------------ driver
def kernel(frames, t_frames, pos_thres, neg_thres):
    frames = np.asarray(frames, dtype=np.float32)
    pos_thres = np.asarray(pos_thres, dtype=np.float32)
    neg_thres = np.asarray(neg_thres, dtype=np.float32)

    nc = _get_nc()
    dg = _diag_host()
    ey = _eyes_host()
    in_maps = []
    for c in range(N_CORES):
        b = c // 2
        hs = slice((c % 2) * HSH, (c % 2 + 1) * HSH)
        in_maps.append(
            {
                "frames": np.ascontiguousarray(frames[b, :, hs, :]),
                "pos": np.ascontiguousarray(pos_thres[b, 0, hs, :]),
                "neg": np.ascontiguousarray(neg_thres[b, 0, hs, :]),
                "diag": dg,
                "eyes": ey,
            }
        )

    res = bass_utils.run_bass_kernel_spmd(nc, in_maps, core_ids=list(range(N_CORES)))
    out = np.empty((B, NUM_BINS, H, W), dtype=np.float32)
    for c in range(N_CORES):
        b = c // 2
        hs = slice((c % 2) * HSH, (c % 2 + 1) * HSH)
        out[b, :, hs, :] = res.results[c]["vox"]
    return out
